# revision 3
# baseline (speedup 1.0000x reference)
"""CapsuleLayer forward (squash + per-capsule matmul) on 8 Trainium2 cores.

Reference computation (all fp32):
    x  = inputs.reshape(B, 1152, 8)
    pc = squash(x)                              # per-(b,n) over k=8
    u_hat[b,n,j,d] = sum_k W[0,n,j,d,k] * pc[b,n,k]
    out = u_hat[..., None]                      # [B, 1152, 10, 16, 1]

Sharding: capsule dim (n=1152) split 144-per-core across 8 cores; every core
keeps the full batch (B=512).  Zero cross-device communication.

Per-core kernel (fp16 data paths; PSUM accumulates fp32):
  - squash scale simplified algebraically: sq/((1+sq)*sqrt(sq+eps))
    == sqrt(sq)/(1+sq) (eps negligible, NaN-safe at sq=0); short DVE chain
    plus one ACT sqrt, no GpSimd in the dependency path
  - W loaded COMPACT ([128, 9*160] fp16, 368KB instead of the 5.9MB padded
    block-diagonal form) and expanded on-chip into 9 resident block-diag
    [128, 16*160] tiles via broadcast-multiply with a [128,16] 0/1 mask:
    DVE handles the early groups (staggered ahead of the PE), GpSimd the
    late ones
  - all 4 x-chunk DMAs issued up-front (only 1.18MB); chunk i+1's scale
    chain emitted mid-chunk i with inputs already resident, so no engine
    queue ever head-of-line blocks on a DMA
  - pc transposed to [ck, b] via PE transpose, pipelined one group ahead
  - matmul out[b, (c,jd)] = pcT.T @ wblk  (K=128, M=128, fp16 operands);
    PE kept continuously busy so it ramps from the 1.2GHz mid p-state to
    the full 2.4GHz
  - PSUM->SBUF evacuation split DVE/ACT (pa->DVE first since the next
    group's first matmul reuses that bank pair); output stored per group
    in [128, 2560] fp16 tiles, alternating the sync/scalar HWDGE rings so
    the SDMA engines round-robin two queue rows and hide per-DMA
    completion-receipt bubbles
"""

from contextlib import ExitStack

import numpy as np

import concourse.bacc as bacc
import concourse.bass as bass  # noqa: F401  (AP helpers)
import concourse.mybir as mybir
import concourse.tile as tile
from concourse.bass_utils import run_bass_kernel_spmd
from concourse.masks import make_identity

N_CORES = 8
B = 512
N_CAPS = 1152
K = 8
JD = 160  # 10*16
CAPS_PER_CORE = N_CAPS // N_CORES  # 144
GROUP_CAPS = 16  # caps per matmul group -> K=128
N_GROUPS = CAPS_PER_CORE // GROUP_CAPS  # 9
GROUP_COLS = GROUP_CAPS * JD  # 2560
P = 128
B_CHUNKS = B // P  # 4
N_DVE_EXP = 5  # wblk groups expanded on DVE (rest on GpSimd)

F32 = mybir.dt.float32
F16 = mybir.dt.float16
OUT_DT = mybir.dt.float16
OUT_NP = np.float16


def build_program():
    nc = bacc.Bacc("TRN2", debug=False, num_devices=N_CORES)
    x = nc.dram_tensor("x", [B, CAPS_PER_CORE * K], F16, kind="ExternalInput").ap()
    wc = nc.dram_tensor(
        "wc", [P, N_GROUPS * JD], F16, kind="ExternalInput"
    ).ap()
    mk = nc.dram_tensor("mk", [P, GROUP_CAPS], F16, kind="ExternalInput").ap()
    out = nc.dram_tensor(
        "out", [B, CAPS_PER_CORE * JD], OUT_DT, kind="ExternalOutput"
    ).ap()

    with tile.TileContext(nc) as tc, ExitStack() as ctx:
        consts = ctx.enter_context(tc.tile_pool(name="consts", bufs=1))
        wblk_pool = ctx.enter_context(tc.tile_pool(name="wblk", bufs=1))
        xpool = ctx.enter_context(tc.tile_pool(name="xpool", bufs=4))
        x2pool = ctx.enter_context(tc.tile_pool(name="x2pool", bufs=2))
        pcpool = ctx.enter_context(tc.tile_pool(name="pcpool", bufs=2))
        stats = ctx.enter_context(tc.tile_pool(name="stats", bufs=2))
        pct_pool = ctx.enter_context(tc.tile_pool(name="pct", bufs=3))
        ost_pool = ctx.enter_context(tc.tile_pool(name="ost", bufs=4))
        # PSUM: 3x 2-bank matmul slots + 2x 1-bank transpose slots = 8 banks.
        psum = ctx.enter_context(tc.tile_pool(name="psum", bufs=3, space="PSUM"))
        psum_t = ctx.enter_context(tc.tile_pool(name="psum_t", bufs=2, space="PSUM"))

        # x chunk 0 goes out first (its squash chain is the critical path),
        # then the compact W + mask, then the remaining x chunks.
        xts = []
        xt0 = xpool.tile([P, CAPS_PER_CORE, K], F16, tag="xt0")
        nc.scalar.dma_start(
            out=xt0, in_=x[0:P, :].rearrange("b (c k) -> b c k", k=K)
        )
        xts.append(xt0)
        wcomp = consts.tile([P, N_GROUPS, JD], F16)
        nc.scalar.dma_start(
            out=wcomp, in_=wc.rearrange("p (g d) -> p g d", d=JD)
        )
        mask = consts.tile([P, GROUP_CAPS], F16)
        nc.scalar.dma_start(out=mask, in_=mk)
        for bi in range(1, B_CHUNKS):
            xt = xpool.tile([P, CAPS_PER_CORE, K], F16, tag=f"xt{bi}")
            nc.scalar.dma_start(
                out=xt,
                in_=x[bi * P : (bi + 1) * P, :].rearrange("b (c k) -> b c k", k=K),
            )
            xts.append(xt)

        identity = consts.tile([P, P], F16)
        make_identity(nc, identity)

        wblk = [
            wblk_pool.tile(
                [P, GROUP_CAPS, JD], F16, tag=f"wblk{g}", name=f"wblk{g}"
            )
            for g in range(N_GROUPS)
        ]
        mask_b = mask.unsqueeze(2).broadcast_to([P, GROUP_CAPS, JD])

        def emit_expand(g, eng):
            # wblk[g][(c,k), (c',jd)] = wcomp[(c,k), g, jd] * (c == c')
            eng.tensor_mul(
                wblk[g],
                wcomp[:, g : g + 1, :].broadcast_to([P, GROUP_CAPS, JD]),
                mask_b,
            )

        # GpSimd is slow (~4.6us/group) but idle: give it the LAST groups
        # in need-order so it stays off the critical path.
        for g in range(N_DVE_EXP, N_GROUPS):
            emit_expand(g, nc.gpsimd)

        def emit_chain(xt, pc):
            # scale[b,c] = sqrt(sq)/(1+sq), pc = x*scale  (fp16 throughout)
            x2 = x2pool.tile([P, CAPS_PER_CORE, K], F16, tag="x2")
            nc.vector.tensor_mul(x2, xt, xt)
            sq = stats.tile([P, CAPS_PER_CORE], F16, tag="sq")
            nc.vector.reduce_sum(out=sq, in_=x2, axis=mybir.AxisListType.X)
            sn = stats.tile([P, CAPS_PER_CORE], F16, tag="sn")
            nc.scalar.activation(
                out=sn, in_=sq, func=mybir.ActivationFunctionType.Sqrt
            )
            t1 = stats.tile([P, CAPS_PER_CORE], F16, tag="t1")
            nc.vector.tensor_scalar_add(t1, sq, 1.0)
            rden = stats.tile([P, CAPS_PER_CORE], F16, tag="rd")
            nc.vector.reciprocal(rden, t1)
            scale = stats.tile([P, CAPS_PER_CORE], F16, tag="sc")
            nc.vector.tensor_mul(scale, sn, rden)
            nc.vector.tensor_mul(
                pc, xt, scale.unsqueeze(2).broadcast_to([P, CAPS_PER_CORE, K])
            )

        def issue_transpose(pc_flat, g):
            # Pipelined one group ahead so the PE never waits on the
            # PSUM->SBUF pcT copy.
            pst = psum_t.tile([P, P], F16, tag="pt")
            nc.tensor.transpose(pst, pc_flat[:, g * P : (g + 1) * P], identity)
            pcT = pct_pool.tile([P, P], F16)
            if g % 2 == 0:
                nc.scalar.copy(pcT, pst)
            else:
                nc.vector.tensor_copy(pcT, pst)
            return pcT

        with nc.allow_low_precision("fp16 squash: tolerance is 2e-2"):
            pc0 = pcpool.tile([P, CAPS_PER_CORE, K], F16, tag="pc")
            emit_chain(xts[0], pc0)
            # DVE expansion for the early groups, staggered two ahead of
            # the PE from inside the chunk-0 loop.
            emit_expand(0, nc.vector)
            emit_expand(1, nc.vector)

            pc_cur = pc0.rearrange("p c k -> p (c k)")
            pc_next = None
            for bi in range(B_CHUNKS):
                pcT_next = issue_transpose(pc_cur, 0)
                for g in range(N_GROUPS):
                    pcT = pcT_next
                    if g + 1 < N_GROUPS:
                        pcT_next = issue_transpose(pc_cur, g + 1)

                    if bi == 0 and g + 2 < N_DVE_EXP:
                        emit_expand(g + 2, nc.vector)

                    if g == 4 and bi + 1 < B_CHUNKS:
                        # Next chunk's scale chain: inputs already resident,
                        # so these DVE/ACT ops never stall their queues.
                        pcn = pcpool.tile([P, CAPS_PER_CORE, K], F16, tag="pc")
                        emit_chain(xts[bi + 1], pcn)
                        pc_next = pcn.rearrange("p c k -> p (c k)")

                    pa = psum.tile([P, 1024], F32, tag="pm")
                    pb = psum.tile([P, 1024], F32, tag="pm")
                    pcs = psum.tile([P, 512], F32, tag="pm")
                    wb = wblk[g].rearrange("p c d -> p (c d)")
                    for s in range(2):
                        nc.tensor.matmul(
                            pa[:, s * 512 : (s + 1) * 512],
                            lhsT=pcT,
                            rhs=wb[:, s * 512 : (s + 1) * 512],
                            start=True,
                            stop=True,
                        )
                    for s in range(2):
                        nc.tensor.matmul(
                            pb[:, s * 512 : (s + 1) * 512],
                            lhsT=pcT,
                            rhs=wb[:, (2 + s) * 512 : (3 + s) * 512],
                            start=True,
                            stop=True,
                        )
                    nc.tensor.matmul(
                        pcs, lhsT=pcT, rhs=wb[:, 4 * 512 : 5 * 512],
                        start=True, stop=True,
                    )

                    ost = ost_pool.tile([P, GROUP_COLS], OUT_DT)
                    # pa evacuates first: the next group's first matmul
                    # reuses its bank pair.  Split DVE/ACT.
                    nc.vector.tensor_copy(ost[:, 0:1024], pa)
                    nc.scalar.copy(ost[:, 1024:2048], pb)
                    if g % 2 == 0:
                        nc.vector.tensor_copy(ost[:, 2048:2560], pcs)
                    else:
                        nc.scalar.copy(ost[:, 2048:2560], pcs)
                    # Alternate the two HWDGE rings per store.
                    st_eng = nc.sync if (bi * N_GROUPS + g) % 2 == 0 else nc.scalar
                    st_eng.dma_start(
                        out=out[
                            bi * P : (bi + 1) * P,
                            g * GROUP_COLS : (g + 1) * GROUP_COLS,
                        ],
                        in_=ost,
                    )
                pc_cur = pc_next
    nc.compile()
    return nc


_PROGRAM = None


def _get_program():
    global _PROGRAM
    if _PROGRAM is None:
        _PROGRAM = build_program()
    return _PROGRAM


def shard_inputs(inputs: np.ndarray, W: np.ndarray) -> list[dict[str, np.ndarray]]:
    # Compact k-major W per core: wc[(c,k), (g,jd)] = W[0, core*144+g*16+c, jd, k]
    # (jd = j*16+d).  The kernel expands this to block-diagonal on-chip.
    w0 = np.asarray(W[0], dtype=np.float32).reshape(N_CAPS, JD, K)
    x16 = np.asarray(inputs, dtype=np.float16)
    mask = np.zeros((P, GROUP_CAPS), dtype=np.float16)
    for c in range(GROUP_CAPS):
        mask[c * K : (c + 1) * K, c] = 1.0
    in_maps = []
    for i in range(N_CORES):
        c0 = i * CAPS_PER_CORE
        wcore = w0[c0 : c0 + CAPS_PER_CORE]  # [144, 160, 8]
        wcomp = (
            wcore.reshape(N_GROUPS, GROUP_CAPS, JD, K)
            .transpose(1, 3, 0, 2)  # [c, k, g, jd]
            .reshape(P, N_GROUPS * JD)
            .astype(np.float16)
        )
        in_maps.append(
            {
                "x": np.ascontiguousarray(
                    x16[:, c0 * K : (c0 + CAPS_PER_CORE) * K]
                ),
                "wc": np.ascontiguousarray(wcomp),
                "mk": mask,
            }
        )
    return in_maps


def unshard_output(results: list[dict[str, np.ndarray]]) -> np.ndarray:
    full = np.empty((B, N_CAPS, JD), dtype=np.float32)
    for i in range(N_CORES):
        c0 = i * CAPS_PER_CORE
        full[:, c0 : c0 + CAPS_PER_CORE, :] = results[i]["out"].reshape(
            B, CAPS_PER_CORE, JD
        ).astype(np.float32)
    return full.reshape(B, N_CAPS, 10, 16, 1)


def kernel(inputs: np.ndarray, W: np.ndarray) -> np.ndarray:
    nc = _get_program()
    in_maps = shard_inputs(np.asarray(inputs), np.asarray(W))
    res = run_bass_kernel_spmd(nc, in_maps, core_ids=list(range(N_CORES)))
    return unshard_output(res.results)
